# revision 1
# baseline (speedup 1.0000x reference)
"""JointLoss Trainium2 kernel.

Math (see reference):
  loss_pos[i] = ||f_i - agents[l_i]||^2                       (exact, fp32)
  neg[i]      = mean over masked j of relu(1 - dist[i,j])
  dist[i,j]   = f2[i] + a2[j] - 2 F@A.T
  answer      = (sum loss_pos + sum neg_src + sum neg_tgt) / (B + n_valid)

Device strategy (per core, 2048 rows, data-parallel over B):
  PE   : v = 2*F@A.T - a2  (bf16 matmul + K=1 rank-1 update) -> PSUM
  ACT  : h = relu(v + (1 - f2)[i])  (per-partition bias) PSUM->SBUF bf16
  DVE  : r = (sim > 0.5)  {0,1} bf16; cnt = reduce_add(r)  (exact integers)
  DVE  : w = min(h, r)  (= masked hinge, h <= 1); sw = reduce_add(w)
  label term removed per-row via exact correction using agents[l] / sim[i,l];
  per-core partials [term_sum, n_valid] combined on host.
(tensor_tensor_reduce and activation accum_out crash this runtime's HW path —
 verified by isolation probes — hence the separate TT + tensor_reduce ops.)
"""

import os
from contextlib import ExitStack

import numpy as np

B, C, D = 16384, 4000, 128
NCORES = 8
BS = B // NCORES  # 2048 rows per core
NIB = BS // 128  # 16 row blocks per core
NSTREAM = 2  # src, tgt
BIG = 2.0**100
INV_BIG = 2.0**-100
PCHUNKS = [(0, 2048), (2048, 4000)]  # psum j-chunks

_CACHE = {}


def _build_nc():
    import concourse.bacc as bacc
    import concourse.tile as tile
    from concourse import mybir
    from concourse.masks import make_identity

    f32 = mybir.dt.float32
    bf16 = mybir.dt.bfloat16
    Alu = mybir.AluOpType
    Act = mybir.ActivationFunctionType
    X = mybir.AxisListType.X

    nc = bacc.Bacc(
        "TRN2",
        target_bir_lowering=False,
        debug=False,
        enable_asserts=False,
        num_devices=NCORES,
    )

    f_d = nc.dram_tensor("f", (BS, D), f32, kind="ExternalInput").ap()
    ft_d = nc.dram_tensor("ft", (BS, D), f32, kind="ExternalInput").ap()
    ag_d = nc.dram_tensor("ag", (C, D), f32, kind="ExternalInput").ap()
    al_d = nc.dram_tensor("al", (BS, D), f32, kind="ExternalInput").ap()
    sim_d = nc.dram_tensor("sim", (BS, C), f32, kind="ExternalInput").ap()
    simt_d = nc.dram_tensor("simt", (BS, C), f32, kind="ExternalInput").ap()
    slbl_d = nc.dram_tensor("slbl", (BS,), f32, kind="ExternalInput").ap()
    out_d = nc.dram_tensor("out", (1, 2), f32, kind="ExternalOutput").ap()

    with tile.TileContext(nc) as tc, ExitStack() as ctx:
        const = ctx.enter_context(tc.tile_pool(name="const", bufs=1))
        setup = ctx.enter_context(tc.tile_pool(name="setup", bufs=3))
        work = ctx.enter_context(tc.tile_pool(name="work", bufs=2))
        scrp = ctx.enter_context(tc.tile_pool(name="scr", bufs=1))
        psum = ctx.enter_context(tc.tile_pool(name="psum", bufs=2, space="PSUM"))

        ident = const.tile([128, 128], f32)
        make_identity(nc, ident)
        ones_col = const.tile([128, 1], f32)
        nc.vector.memset(ones_col, 1.0)
        ones_row_bf = const.tile([1, 128], bf16)
        nc.vector.memset(ones_row_bf, 1.0)
        neg_half = const.tile([128, 1], f32)
        nc.vector.memset(neg_half, -0.5)

        # persistent per-core state
        agT2 = const.tile([128, C], bf16)  # 2 * A.T
        nega2 = const.tile([1, C], bf16)  # -a2 row
        ftT = const.tile([128, NSTREAM * BS], bf16)  # [F.T | FT.T]
        bias_st = const.tile([128, NSTREAM * NIB], f32)  # 1 - f2
        cnt_st = const.tile([128, NSTREAM * NIB], f32)  # scaled counts
        sw_st = const.tile([128, NSTREAM * NIB], f32)  # hinge sums
        lp_st = const.tile([128, NIB], f32)  # loss_pos cols
        ml_st = const.tile([128, NIB], f32)  # (sim[i,l] > .5)

        # ---- agents setup: transpose + 2x scale + squared-norm row ----
        agsq = setup.tile([128, C], f32, tag="agsq")
        for jb in range(32):
            rows = min(128, C - jb * 128)
            ablk = setup.tile([128, 128], f32, tag="ablk")
            if rows < 128:
                nc.vector.memset(ablk, 0.0)
            nc.sync.dma_start(
                out=ablk[:rows, :], in_=ag_d[jb * 128 : jb * 128 + rows, :]
            )
            pst = psum.tile([128, 2048], f32, tag="ps")
            nc.tensor.transpose(pst[:, :128], ablk, ident)
            nc.scalar.activation(
                out=agT2[:, jb * 128 : jb * 128 + rows],
                in_=pst[:, :rows],
                func=Act.Copy,
                scale=2.0,
            )
            nc.scalar.activation(
                out=agsq[:, jb * 128 : jb * 128 + rows],
                in_=pst[:, :rows],
                func=Act.Square,
            )
        for k in range(8):
            n = min(512, C - k * 512)
            psa = psum.tile([128, 2048], f32, tag="ps")
            nc.tensor.matmul(
                psa[0:1, :n],
                lhsT=ones_col,
                rhs=agsq[:, k * 512 : k * 512 + n],
                start=True,
                stop=True,
            )
            nc.scalar.activation(
                out=nega2[0:1, k * 512 : k * 512 + n],
                in_=psa[0:1, :n],
                func=Act.Copy,
                scale=-1.0,
            )

        # ---- features setup (both streams) ----
        for s, src in enumerate([f_d, ft_d]):
            for ib in range(NIB):
                fblk = setup.tile([128, D], f32, tag="fblk")
                nc.sync.dma_start(out=fblk, in_=src[ib * 128 : (ib + 1) * 128, :])
                scr = setup.tile([128, D], f32, tag="fscr")
                nc.vector.tensor_tensor(out=scr, in0=fblk, in1=fblk, op=Alu.mult)
                nc.vector.tensor_reduce(
                    bias_st[:, s * NIB + ib : s * NIB + ib + 1], scr, axis=X, op=Alu.add
                )
                pst = psum.tile([128, 2048], f32, tag="ps")
                nc.tensor.transpose(pst[:, :128], fblk, ident)
                col = s * BS + ib * 128
                nc.scalar.activation(
                    out=ftT[:, col : col + 128], in_=pst[:, :128], func=Act.Copy
                )
                if s == 0:
                    alblk = setup.tile([128, D], f32, tag="alblk")
                    nc.sync.dma_start(
                        out=alblk, in_=al_d[ib * 128 : (ib + 1) * 128, :]
                    )
                    dblk = setup.tile([128, D], f32, tag="dblk")
                    nc.vector.tensor_tensor(
                        out=dblk, in0=fblk, in1=alblk, op=Alu.subtract
                    )
                    scr2 = setup.tile([128, D], f32, tag="fscr2")
                    nc.vector.tensor_tensor(out=scr2, in0=dblk, in1=dblk, op=Alu.mult)
                    nc.vector.tensor_reduce(
                        lp_st[:, ib : ib + 1], scr2, axis=X, op=Alu.add
                    )
        # bias = 1 - f2 (in place over the f2 accumulators)
        nc.scalar.activation(
            out=bias_st, in_=bias_st, func=Act.Copy, scale=-1.0, bias=1.0
        )
        # sim at label + its mask column
        slbl_t = setup.tile([128, NIB], f32, tag="slbl")
        nc.sync.dma_start(out=slbl_t, in_=slbl_d.rearrange("(b p) -> p b", p=128))
        nc.vector.tensor_scalar(ml_st, slbl_t, 0.5, None, Alu.is_gt)

        # ---- main loop ----
        for s, simsrc in enumerate([sim_d, simt_d]):
            for ib in range(NIB):
                sc = s * NIB + ib
                sim_t = work.tile([128, C], f32, tag="sim")
                nc.sync.dma_start(
                    out=sim_t, in_=simsrc[ib * 128 : (ib + 1) * 128, :]
                )
                r_t = work.tile([128, C], bf16, tag="r")
                nc.vector.tensor_scalar(r_t, sim_t, 0.5, None, Alu.is_gt)
                nc.vector.tensor_reduce(
                    cnt_st[:, sc : sc + 1], r_t, axis=X, op=Alu.add
                )
                h_t = work.tile([128, C], bf16, tag="h")
                for js, je in PCHUNKS:
                    pv = psum.tile([128, 2048], f32, tag="ps")
                    for k in range(js, je, 512):
                        n = min(512, je - k)
                        nc.tensor.matmul(
                            pv[:, k - js : k - js + n],
                            lhsT=ftT[:, s * BS + ib * 128 : s * BS + (ib + 1) * 128],
                            rhs=agT2[:, k : k + n],
                            start=True,
                            stop=False,
                        )
                        nc.tensor.matmul(
                            pv[:, k - js : k - js + n],
                            lhsT=ones_row_bf,
                            rhs=nega2[0:1, k : k + n],
                            start=False,
                            stop=True,
                        )
                    nc.scalar.activation(
                        out=h_t[:, js:je],
                        in_=pv[:, : je - js],
                        func=Act.Relu,
                        bias=bias_st[:, sc : sc + 1],
                    )
                w_t = scrp.tile([128, C], bf16, tag="w")
                nc.vector.tensor_tensor(out=w_t, in0=h_t, in1=r_t, op=Alu.min)
                nc.vector.tensor_reduce(
                    sw_st[:, sc : sc + 1], w_t, axis=X, op=Alu.add
                )

        # ---- finalize ----
        fin = ctx.enter_context(tc.tile_pool(name="fin", bufs=1))
        cntf = cnt_st  # counts are exact integers already
        # src label corrections
        hl = fin.tile([128, NIB], f32)
        nc.scalar.activation(out=hl, in_=lp_st, func=Act.Relu, scale=-1.0, bias=ones_col)
        corr = fin.tile([128, NIB], f32)
        nc.vector.tensor_tensor(out=corr, in0=hl, in1=ml_st, op=Alu.mult)
        nc.vector.tensor_tensor(
            out=sw_st[:, :NIB], in0=sw_st[:, :NIB], in1=corr, op=Alu.subtract
        )
        nc.vector.tensor_tensor(
            out=cntf[:, :NIB], in0=cntf[:, :NIB], in1=ml_st, op=Alu.subtract
        )
        # neg = sw / max(cnt, 1); valid = cnt > 0
        den = fin.tile([128, NSTREAM * NIB], f32)
        nc.vector.tensor_scalar(den, cntf, 1.0, None, Alu.max)
        rec = fin.tile([128, NSTREAM * NIB], f32)
        nc.vector.reciprocal(rec, den)
        neg = fin.tile([128, NSTREAM * NIB], f32)
        nc.vector.tensor_tensor(out=neg, in0=sw_st, in1=rec, op=Alu.mult)
        valid = fin.tile([128, NSTREAM * NIB], f32)
        nc.vector.tensor_scalar(valid, cntf, 0.0, None, Alu.is_gt)
        # row totals
        tcol = fin.tile([128, 1], f32)
        t2 = fin.tile([128, 1], f32)
        nc.vector.tensor_reduce(tcol, neg, axis=X, op=Alu.add)
        nc.vector.tensor_reduce(t2, lp_st, axis=X, op=Alu.add)
        pack = fin.tile([128, 2], f32)
        nc.vector.tensor_tensor(out=pack[:, 0:1], in0=tcol, in1=t2, op=Alu.add)
        nc.vector.tensor_reduce(pack[:, 1:2], valid, axis=X, op=Alu.add)
        psf = psum.tile([128, 2048], f32, tag="ps")
        nc.tensor.matmul(psf[0:1, 0:2], lhsT=ones_col, rhs=pack, start=True, stop=True)
        outt = fin.tile([1, 2], f32)
        nc.scalar.activation(out=outt, in_=psf[0:1, 0:2], func=Act.Copy)
        nc.sync.dma_start(out=out_d, in_=outt)

    nc.compile()
    return nc


def _get_nc():
    if "nc" not in _CACHE:
        _CACHE["nc"] = _build_nc()
    return _CACHE["nc"]


def make_in_maps(features, agents, labels, similarity, features_target, similarity_target):
    labels = np.asarray(labels).astype(np.int64)
    al_full = np.ascontiguousarray(np.asarray(agents)[labels], dtype=np.float32)
    slbl_full = np.ascontiguousarray(
        np.asarray(similarity)[np.arange(B), labels], dtype=np.float32
    )
    c32 = lambda x: np.ascontiguousarray(x, dtype=np.float32)
    in_maps = []
    for c in range(NCORES):
        r = slice(c * BS, (c + 1) * BS)
        in_maps.append(
            {
                "f": c32(features[r]),
                "ft": c32(features_target[r]),
                "ag": c32(agents),
                "al": al_full[r],
                "sim": c32(similarity[r]),
                "simt": c32(similarity_target[r]),
                "slbl": slbl_full[r],
            }
        )
    return in_maps


def kernel(features, agents, labels, similarity, features_target, similarity_target):
    from concourse import bass_utils

    nc = _get_nc()
    in_maps = make_in_maps(
        features, agents, labels, similarity, features_target, similarity_target
    )
    res = bass_utils.run_bass_kernel_spmd(
        nc, in_maps, core_ids=list(range(NCORES)), trace=False
    )
    _CACHE["last_results"] = res
    parts = np.stack([r["out"][0] for r in res.results])  # [8, 2]
    term_sum = float(parts[:, 0].sum())
    n_valid = float(parts[:, 1].sum())
    return np.float32(term_sum / (B + n_valid))

